# revision 37
# baseline (speedup 1.0000x reference)
"""
Multi-head masked (causal) attention on 8 Trainium2 NeuronCores.

Sharding: core = 2*b + g  (b = batch 0..3, g = head-group 0..1, 6 heads each).
Each core computes, for its batch b and heads [6g, 6g+6):
    q,k,v projections -> causal attention -> partial out-projection
    (rows [384g, 384g+384) of Wo), output written TRANSPOSED [768, S] bf16.
Host gathers: out[b] = (part[2b] + part[2b+1]).T + bo.

All matmuls in bf16 (PE 1 cycle/row vs fp32's 4), fp32 PSUM accumulation.
Scores are computed transposed (S^T[sk, sq] = K^T x Q^T) so:
  - exp runs on ACT straight out of PSUM (scale=1/8 fused),
  - AV uses V as the stationary operand with an appended ones-column,
    yielding ctx^T[j, sq] AND the softmax denominator in one accumulation,
  - ctx^T is exactly the lhsT layout the out-projection needs.
Causal structure is exploited block-exactly: for key-block ik only
sq >= 128*ik is computed; diagonal 128x128 blocks get -30000 added to the
masked entries via one I.T @ mneg PE matmul before the exp.

DMA: weights stream on the sync queue in first-use order (wq/wk pair 0
first), x^T streams on the scalar queue in 512-column quarters, so the
first projection matmul starts as soon as quarter 0 + pair-0 weights land
instead of waiting for the whole input.

Out-projection accumulates all 3 head-pairs into one PSUM tile and stores
a single bf16 [768, S] output (t2=0 fully overlapped with attention; for
t2=1 the pair-0/1 partial is staged to SBUF during the last attention call
and only the pair-2 matmul + add + store remain in the tail).
"""

import numpy as np
import ml_dtypes

import concourse.bass as bass
import concourse.mybir as mybir
import concourse.tile as tile
from concourse import bacc

BF16 = mybir.dt.bfloat16
F32 = mybir.dt.float32

# Problem constants (hardcoded per contract)
B, S, D = 4, 2048, 768
N_HEADS_TOTAL = 12
HD = 64                      # head dim
H = 6                        # local heads per core
NPAIR = H // 2               # head pairs (Q/K computed 2 heads at a time)
NC_D = D // 128              # contraction chunks over D (6)
NSK = S // 128               # key blocks (16)
BT = 1024                    # query-tile width for the attention phase
NT2 = S // BT                # query tiles (2)
VW = H * (HD + 64)           # v storage: per head [v(64) | ones(64)] (768)
SCALE = 1.0 / np.sqrt(HD)


def _chunks(total, step=512):
    out = []
    n0 = 0
    while n0 < total:
        w = min(step, total - n0)
        out.append((n0, w))
        n0 += w
    return out


def build_nc():
    nc = bacc.Bacc(None, target_bir_lowering=False)

    xT_d = nc.declare_dram_parameter("xT", [D, S], BF16, isOutput=False)
    # per-pair packed: rows [p*128,(p+1)*128) are pair p as [r, (c m)]
    wq_d = nc.declare_dram_parameter("wq", [NPAIR * 128, NC_D * 128], BF16,
                                     isOutput=False)
    wk_d = nc.declare_dram_parameter("wk", [NPAIR * 128, NC_D * 128], BF16,
                                     isOutput=False)
    wv_d = nc.declare_dram_parameter("wv", [128, NC_D * H * HD], BF16,
                                     isOutput=False)
    bq_d = nc.declare_dram_parameter("bq", [128, NPAIR], F32, isOutput=False)
    bk_d = nc.declare_dram_parameter("bk", [128, NPAIR], F32, isOutput=False)
    bv_d = nc.declare_dram_parameter("bv", [1, H * HD], BF16, isOutput=False)
    wo_d = nc.declare_dram_parameter("wo", [128, NPAIR * D], BF16,
                                     isOutput=False)
    # tri[sk, sq] = 1 where sq >= sk: multiplied into the exp'd diagonal
    # S^T block on DVE (cheaper than a PE mask matmul)
    tri_d = nc.declare_dram_parameter("tri", [128, 128], BF16,
                                      isOutput=False)
    ident_d = nc.declare_dram_parameter("ident", [128, 128], BF16,
                                        isOutput=False)
    outT_d = nc.declare_dram_parameter("outT", [D, S], BF16, isOutput=True)

    with tile.TileContext(nc) as tc:
        with (
            tc.tile_pool(name="const", bufs=1) as constp,
            tc.tile_pool(name="big", bufs=1) as bigp,
            tc.tile_pool(name="epool", bufs=4) as epool,
            tc.tile_pool(name="rpool", bufs=2) as rpool,
            tc.tile_pool(name="opool", bufs=3) as opool,
            tc.tile_pool(name="work", bufs=2, space="PSUM") as work,
            tc.tile_pool(name="ctxp", bufs=2, space="PSUM") as ctxp,
        ):
            # ---- x^T on the scalar HWDGE queue, in 256-col chunks so the
            # first projection tiles gate on as little input as possible
            xT_sb = bigp.tile([128, NC_D, S], BF16)
            xT_r = xT_d.rearrange("(c p) s -> p c s", p=128)
            for e in range(8):
                nc.scalar.dma_start(
                    xT_sb[:, :, e * 256:(e + 1) * 256],
                    xT_r[:, :, e * 256:(e + 1) * 256],
                )
            # preload the Exp activation table while DMAs stream (issued
            # after the xT DMAs so it doesn't delay them on this queue)
            scr0 = constp.tile([128, 1], F32)
            scr1 = constp.tile([128, 1], F32)
            nc.vector.memset(scr0[:], 0.0)
            nc.scalar.activation(scr1[:], scr0[:],
                                 mybir.ActivationFunctionType.Exp, scale=1.0)
            # ---- weights on the sync HWDGE queue, in first-use order
            wq_sb = constp.tile([128, NPAIR, NC_D, 128], BF16)
            wk_sb = constp.tile([128, NPAIR, NC_D, 128], BF16)
            nc.sync.dma_start(wq_sb[:, 0], wq_d[0:128, :])
            nc.sync.dma_start(wk_sb[:, 0], wk_d[0:128, :])
            bq_sb = constp.tile([128, NPAIR], F32)
            nc.sync.dma_start(bq_sb[:], bq_d[:])
            bk_sb = constp.tile([128, NPAIR], F32)
            nc.sync.dma_start(bk_sb[:], bk_d[:])
            tri_sb = constp.tile([128, 128], BF16)
            nc.sync.dma_start(tri_sb[:], tri_d[:])
            ident_sb = constp.tile([128, 128], BF16)
            nc.sync.dma_start(ident_sb[:], ident_d[:])
            wv_sb = constp.tile([128, NC_D, H * HD], BF16)
            nc.sync.dma_start(wv_sb[:].rearrange("p c n -> p (c n)"), wv_d[:])
            bv_sb = constp.tile([1, H * HD], BF16)
            nc.sync.dma_start(bv_sb[:], bv_d[:])
            ones1_sb = constp.tile([1, 128], BF16)
            nc.vector.memset(ones1_sb[:], 1.0)
            for p in (1, 2):
                nc.sync.dma_start(wq_sb[:, p], wq_d[p * 128:(p + 1) * 128, :])
                nc.sync.dma_start(wk_sb[:, p], wk_d[p * 128:(p + 1) * 128, :])
            # wo is needed only ~100us in: stream it after xT on the scalar
            # queue so it doesn't compete with the startup-critical input
            wo_sb = constp.tile([128, NPAIR, D], BF16)
            nc.scalar.dma_start(wo_sb[:].rearrange("p c n -> p (c n)"), wo_d[:])

            qT_sb = bigp.tile([128, NPAIR, S], BF16)
            kT_sb = bigp.tile([128, NPAIR, S], BF16)
            v_sb = bigp.tile([128, NSK, VW], BF16)
            ctxT_sb = bigp.tile([128, NPAIR, S], BF16)
            stage_sb = bigp.tile([128, D // 128, 512], BF16)
            # ones-blocks of v (cols [64,128) per head) — set once on DVE
            nc.vector.memset(
                v_sb[:].rearrange("p s (h c) -> p s h c", h=H)[:, :, :, HD:128],
                1.0,
            )

            def qk_tile(p, which, t, n0=0, nw=512):
                dst_sb, w_sb, b_sb = ((qT_sb, wq_sb, bq_sb),
                                      (kT_sb, wk_sb, bk_sb))[which]
                ps = work.tile([128, 1024], F32, tag="work")
                for c in range(NC_D):
                    nc.tensor.matmul(
                        ps[:, 0:nw],
                        w_sb[:, p, c, :],
                        xT_sb[:, c, t * 512 + n0:t * 512 + n0 + nw],
                        start=(c == 0), stop=(c == NC_D - 1),
                    )
                nc.vector.tensor_add(
                    out=dst_sb[:, p, t * 512 + n0:t * 512 + n0 + nw],
                    in0=ps[:, 0:nw],
                    in1=b_sb[:, p:p + 1].broadcast_to((128, nw)),
                )

            def v_proj(s):
                ps = work.tile([128, 1024], F32, tag="work")
                for c in range(NC_D):
                    nc.tensor.matmul(
                        ps[:, 0:H * HD],
                        xT_sb[:, c, s * 128:(s + 1) * 128],
                        wv_sb[:, c, :],
                        start=(c == 0), stop=False,
                    )
                # bias via K=1 ones-matmul so the PSUM drain is a plain copy
                # that can run on ACT (keeps DVE free for masks/normalization)
                nc.tensor.matmul(
                    ps[:, 0:H * HD], ones1_sb[:], bv_sb[:],
                    start=False, stop=True,
                )
                nc.scalar.copy(
                    v_sb[:, s, :].rearrange("p (h c) -> p h c", h=H)[:, :, 0:HD],
                    ps[:, 0:H * HD].rearrange("p (h c) -> p h c", h=H),
                )

            def attention(p, t2, fillers=None, defer_norm=False,
                          split_stop_ik=None, gated_factory=None):
                # both heads of pair p, interleaved: the two K=64 score
                # matmuls target PE row-groups 0/64 (auto tile_position from
                # lhsT base partition).
                # Software-pipelined: S/exp of ik+1 are emitted BEFORE the AV
                # matmuls of ik so ACT always has a score tile to exp.
                q0 = t2 * BT
                nik = (q0 + BT) // 128
                ctxs = [ctxp.tile([128, BT], F32, tag="ctx", name=f"ctx{p}{t2}{hf}")
                        for hf in range(2)]

                def emit_scores(ik):
                    sq0 = max(q0, 128 * ik)
                    W = q0 + BT - sq0
                    es = []
                    for half in range(2):
                        hp = slice(half * 64, half * 64 + 64)
                        s_ps = work.tile([128, 1024], F32, tag="work",
                                         name=f"sps{half}")
                        for (n0, w) in _chunks(W):
                            nc.tensor.matmul(
                                s_ps[:, n0:n0 + w],
                                kT_sb[hp, p, ik * 128:(ik + 1) * 128],
                                qT_sb[hp, p, sq0 + n0:sq0 + n0 + w],
                                start=True, stop=True,
                                skip_group_check=True,
                            )
                        e_sb = epool.tile([128, 1024], BF16, tag="e",
                                          name=f"e{half}")
                        nc.scalar.activation(
                            e_sb[:, 0:W], s_ps[:, 0:W],
                            mybir.ActivationFunctionType.Exp, scale=float(SCALE),
                        )
                        if 128 * ik >= q0:
                            # diagonal block at cols [0,128): zero the
                            # causally-masked entries (exp'd real scores)
                            nc.vector.tensor_mul(e_sb[:, 0:128],
                                                 e_sb[:, 0:128], tri_sb[:])
                        es.append(e_sb)
                    return es

                gated = gated_factory(ctxs) if gated_factory else {}
                es_next = emit_scores(0)
                for ik in range(nik):
                    es = es_next
                    if ik + 1 < nik:
                        es_next = emit_scores(ik + 1)
                    for f in gated.get(ik, []):
                        f()
                    if fillers:
                        # spread remaining fillers evenly over remaining iks
                        npop = -(-len(fillers) // (nik - ik))
                        for _ in range(npop):
                            fillers.pop(0)()
                    sq0 = max(q0, 128 * ik)
                    for half in range(2):
                        h = 2 * p + half
                        off = sq0 - q0
                        cw, c0 = [], off
                        while c0 < BT:
                            w = min(512 - (c0 % 512), BT - c0)
                            cw.append((c0, w))
                            c0 += w
                        if 128 * ik >= q0 and len(cw) > 1:
                            # diag chunk last: its e-tile also waits on the
                            # DVE mask multiply
                            cw = cw[1:] + cw[:1]
                        for (c0, w) in cw:
                            # columns [0, 512) receive their final block at
                            # split_stop_ik: stop their accumulation group
                            # there so they can be normalized mid-call
                            if split_stop_ik is not None and c0 < 512:
                                stop = (ik == split_stop_ik)
                            else:
                                stop = (ik == nik - 1)
                            nc.tensor.matmul(
                                ctxs[half][:, c0:c0 + w],
                                v_sb[:, ik, h * 128:(h + 1) * 128],
                                es[half][:, c0 - off:c0 - off + w],
                                start=(ik == 0), stop=stop,
                                skip_group_check=True,
                            )
                # normalize: ctx^T /= denom (rows 64:128 hold the denom),
                # straight off PSUM (recip then mul), releasing each half's
                # ctx slot right after its mul.
                if defer_norm:
                    return ctxs
                for half in range(2):
                    hp = slice(half * 64, half * 64 + 64)
                    rs = rpool.tile([64, BT], F32, tag="rs")
                    nc.vector.tensor_copy(rs[:], ctxs[half][64:128, :])
                    rcp = rpool.tile([64, BT], F32, tag="rcp")
                    nc.vector.reciprocal_approx_fast(rcp[:], rs[:])
                    nc.vector.tensor_mul(
                        ctxT_sb[hp, p, q0:q0 + BT],
                        ctxs[half][0:HD, :],
                        rcp[:],
                    )

            def out_tile0(n, col0, w, ceng="v"):
                # all three pair-chunks into one PSUM tile, bf16 store
                po = work.tile([128, 1024], F32, tag="work")
                for c in range(NPAIR):
                    nc.tensor.matmul(
                        po[:, 0:w],
                        wo_sb[:, c, n * 128:(n + 1) * 128],
                        ctxT_sb[:, c, col0:col0 + w],
                        start=(c == 0), stop=(c == NPAIR - 1),
                    )
                ot = opool.tile([128, 512], BF16, tag="ot")
                if ceng == "v" or (ceng == "alt" and n % 2 == 0):
                    nc.vector.tensor_copy(ot[:, 0:w], po[:, 0:w])
                else:
                    nc.scalar.copy(ot[:, 0:w], po[:, 0:w])
                nc.sync.dma_start(outT_d[n * 128:(n + 1) * 128, col0:col0 + w],
                                  ot[:, 0:w])

            def out_stage01(n, n0, w):
                # t2=1 pairs 0+1 partial -> bf16 SBUF stage (runs inside the
                # last attention call); copies alternate DVE/ACT
                po = work.tile([128, 1024], F32, tag="work")
                for c in (0, 1):
                    nc.tensor.matmul(
                        po[:, 0:w],
                        wo_sb[:, c, n * 128:(n + 1) * 128],
                        ctxT_sb[:, c, BT + n0:BT + n0 + w],
                        start=(c == 0), stop=(c == 1),
                    )
                if n % 2 == 0:
                    nc.vector.tensor_copy(stage_sb[:, n, 0:w], po[:, 0:w])
                else:
                    nc.scalar.copy(stage_sb[:, n, 0:w], po[:, 0:w])

            def out_final2(n, n0, w):
                # tail: pair-2 matmul + staged partial re-added on the PE via
                # an identity matmul, so the drain is a copy (ACT/DVE split)
                po = work.tile([128, 1024], F32, tag="work")
                nc.tensor.matmul(
                    po[:, 0:w],
                    wo_sb[:, 2, n * 128:(n + 1) * 128],
                    ctxT_sb[:, 2, BT + n0:BT + n0 + w],
                    start=True, stop=False,
                )
                nc.tensor.matmul(
                    po[:, 0:w], ident_sb[:], stage_sb[:, n, 0:w],
                    start=False, stop=True,
                )
                ot = opool.tile([128, 512], BF16, tag="ot")
                if n % 2 == 0:
                    nc.vector.tensor_copy(ot[:, 0:w], po[:, 0:w])
                else:
                    nc.scalar.copy(ot[:, 0:w], po[:, 0:w])
                eng = nc.sync if n % 2 == 0 else nc.scalar
                eng.dma_start(outT_d[n * 128:(n + 1) * 128,
                                     BT + n0:BT + n0 + w],
                              ot[:, 0:w])

            # ---- emission order -------------------------------------------
            # Prologue: pair-0 Q/K interleaved with the first half of V so the
            # PE tracks the arriving x^T quarters; remaining projections and
            # the out-projection run as fillers inside the attention calls.
            import functools
            for t in range(4):
                qk_tile(0, 0, t)
                qk_tile(0, 1, t)
                v_proj(2 * t)
                v_proj(2 * t + 1)

            def qk_fillers(p):
                # half-width units (~0.64us of PE each) for fine-grained gaps
                return [functools.partial(qk_tile, p, which, t, n0, 256)
                        for which in range(2) for t in range(S // 512)
                        for n0 in (0, 256)]

            vfill = [functools.partial(v_proj, s)
                     for s in range(8, NSK)]
            o0fill = [functools.partial(out_tile0, n, n0, w)
                      for n in range(D // 128) for (n0, w) in _chunks(BT)]
            # t2=1 cols [1536,2048): pair-0/1 partials staged during the last
            # attention call; the pair-2 contribution lands in the tail
            s01fill = [functools.partial(out_stage01, n, 512, 512)
                       for n in range(D // 128)]

            attention(0, 0, qk_fillers(1))
            attention(1, 0, qk_fillers(2) + vfill[:4])
            attention(2, 0, vfill[4:])
            attention(0, 1, o0fill[:6])
            attention(1, 1, o0fill[6:])

            def gated21(ctxs):
                # ctx cols [0,512) of the last call are final after ik=11:
                # normalize them mid-call, then out-project cols [1024,1536)
                # (all three pairs ready) while the call finishes
                def norm2a(half):
                    hp = slice(half * 64, half * 64 + 64)
                    rs = rpool.tile([64, 512], F32, tag="rs2")
                    nc.vector.tensor_copy(rs[:], ctxs[half][64:128, 0:512])
                    rcp = rpool.tile([64, 512], F32, tag="rcp2")
                    nc.vector.reciprocal_approx_fast(rcp[:], rs[:])
                    nc.vector.tensor_mul(
                        ctxT_sb[hp, 2, BT:BT + 512],
                        ctxs[half][0:HD, 0:512],
                        rcp[:],
                    )
                f3 = [functools.partial(out_tile0, n, BT, 512, "alt")
                      for n in range(D // 128)]
                return {12: [functools.partial(norm2a, 0),
                             functools.partial(norm2a, 1)],
                        13: f3[0:2], 14: f3[2:4], 15: f3[4:6]}

            ctxs2 = attention(2, 1, s01fill, defer_norm=True,
                              split_stop_ik=11, gated_factory=gated21)
            # tail: only cols [1536,2048) of pair 2 remain
            for half in range(2):
                hp = slice(half * 64, half * 64 + 64)
                rs = rpool.tile([64, 512], F32, tag="rs2")
                nc.scalar.copy(rs[:], ctxs2[half][64:128, 512:1024])
                rcp = rpool.tile([64, 512], F32, tag="rcp2")
                nc.vector.reciprocal_approx_fast(rcp[:], rs[:])
                nc.vector.tensor_mul(
                    ctxT_sb[hp, 2, BT + 512:S],
                    ctxs2[half][0:HD, 512:1024],
                    rcp[:],
                )
            for n in range(D // 128):
                out_final2(n, 512, 512)
    nc.finalize()
    return nc


_NC_CACHE = None


def _get_nc():
    global _NC_CACHE
    if _NC_CACHE is None:
        _NC_CACHE = build_nc()
    return _NC_CACHE


def make_in_maps(x, Wq, Wk, Wv, bq, bk, bv, Wo, bo):
    bf16 = ml_dtypes.bfloat16
    # tri[sk, sq] = 1 where sq >= sk (keep), 0 on causally-masked entries
    tri = (np.arange(128)[None, :] >= np.arange(128)[:, None]) \
        .astype(np.float32).astype(bf16)
    ident = np.eye(128, dtype=np.float32).astype(bf16)

    def pack_pairs(w_all):
        # [D, 384] -> [NPAIR*128, NC_D*128]: pair p rows = [r, (c m)]
        blks = []
        for p in range(NPAIR):
            blk = w_all[:, p * 128:(p + 1) * 128]          # [768, 128]
            blk = blk.reshape(NC_D, 128, 128).transpose(1, 0, 2)
            blks.append(blk.reshape(128, NC_D * 128))
        return np.ascontiguousarray(np.concatenate(blks, axis=0))

    in_maps = []
    for core in range(8):
        b, g = core // 2, core % 2
        hs = slice(6 * g, 6 * g + 6)
        xT = np.ascontiguousarray(np.asarray(x[b]).T).astype(bf16)
        wq_all = np.asarray(Wq[hs]).transpose(1, 0, 2).reshape(D, H * HD)
        wk_all = np.asarray(Wk[hs]).transpose(1, 0, 2).reshape(D, H * HD)
        wq = pack_pairs(wq_all).astype(bf16)
        wk = pack_pairs(wk_all).astype(bf16)
        wv_all = np.asarray(Wv[hs]).transpose(1, 0, 2).reshape(D, H * HD)
        wv = np.ascontiguousarray(
            wv_all.reshape(NC_D, 128, H * HD).transpose(1, 0, 2)
            .reshape(128, NC_D * H * HD)).astype(bf16)
        bqc = np.zeros((128, NPAIR), np.float32)
        bkc = np.zeros((128, NPAIR), np.float32)
        for p in range(NPAIR):
            bqc[0:64, p] = bq[6 * g + 2 * p]
            bqc[64:128, p] = bq[6 * g + 2 * p + 1]
            bkc[0:64, p] = bk[6 * g + 2 * p]
            bkc[64:128, p] = bk[6 * g + 2 * p + 1]
        bvr = np.ascontiguousarray(
            np.asarray(bv[hs]).reshape(1, H * HD)).astype(bf16)
        wo_slice = np.asarray(Wo[384 * g:384 * (g + 1), :])   # [384, 768]
        wo = np.ascontiguousarray(
            wo_slice.reshape(NPAIR, 128, D).transpose(1, 0, 2)
            .reshape(128, NPAIR * D)).astype(bf16)
        in_maps.append({
            "xT": xT, "wq": wq, "wk": wk, "wv": wv,
            "bq": bqc, "bk": bkc, "bv": bvr, "wo": wo,
            "tri": tri, "ident": ident,
        })
    return in_maps


def gather_out(results, bo):
    out = np.empty((B, S, D), np.float32)
    bo32 = np.asarray(bo, np.float32)
    for b in range(B):
        pT = (results[2 * b]["outT"].astype(np.float32)
              + results[2 * b + 1]["outT"].astype(np.float32))
        out[b] = pT.T + bo32[None, :]
    return out


def kernel(x, Wq, Wk, Wv, bq, bk, bv, Wo, bo):
    from concourse.bass_utils import run_bass_kernel_spmd

    nc = _get_nc()
    in_maps = make_in_maps(x, Wq, Wk, Wv, bq, bk, bv, Wo, bo)
    res = run_bass_kernel_spmd(nc, in_maps, list(range(8)))
    return gather_out(res.results, bo)
